# revision 33
# baseline (speedup 1.0000x reference)
"""MultiHeadCrossAttention kernel for 8 Trainium2 NeuronCores.

Reference computation (b=2, nq=nk=2048, d_model=512, h=8, hd=64):
    Q = split_heads(q @ Wq.T + bq); K, V likewise
    S = Q K^T * hd^-0.5 ; A = softmax(S, -1) * mask_head * diag(pearson)
    out = merge_heads(A @ V)

Sharding: 16 (batch, head) pairs -> 2 heads of one batch per core.

Only the *diagonal* of pearson_matrix is used, so it is extracted on the
host and folded into the mask.  The mask is transposed, diag-folded,
tiled to the exact per-iteration consumption order and cast to bf16 on
the host, so every device-side mask DMA is one contiguous 512 KiB read.

All matmul operands are bf16 (PE streams 1 column/cycle for bf16; fp16
runs at half that rate).  PSUM accumulation stays fp32.  bf16's 8
mantissa bits keep the rel-err ~4e-3, under the 2e-2 gate.

Device layout per core ("k on partitions, q on free axis"):
    S^T[k,q]   = sum_d K^T[d,k] Q^T[d,q]       (TensorE, d=64 contraction,
                                                2 heads run concurrently in
                                                disjoint row halves)
    E^T        = exp(SCALING * S^T)            (ScalarE, PSUM->SBUF bf16)
    Z[q]      += ones^T @ E^T                  (TensorE, PSUM-accumulated,
                                                heads packed in col groups
                                                0/32 -> run concurrently)
    A^T        = E^T * maskT_folded            (VectorE, bf16 2x mode)
    agg^T[e,q]+= sum_k V[k,e] A^T[k,q]         (TensorE, PSUM-accumulated,
                                                heads col-packed 0/64)

The scalar-engine exp stream (~1.12us per [128,1024] tile, 64 tiles) is
the critical path; everything else is scheduled around keeping it dense:

 * k/q/v DRAM staging is per-partition contiguous so DMAs run at full
   rate; the DMA queue order puts qa + the first k-tile of ka ahead of
   everything else so the first S^T matmul (and exp) start ~15us in.
 * The K-A projection is split: k-tile 0 only (gating exp #0), then the
   rest wedged into iteration 1.
 * Z/AV matmul packs are emitted several iterations late (ZLAG/AVLAG) so
   the PE never blocks the S->exp chain on mask/projection stragglers.
 * The K-B/Q-B/V projections are wedged into early iterations, borrowing
   the agg/z PSUM tags before the (lagged) first Z/AV accumulations
   claim them.
 * A burst of tiny warm-up matmuls opens the PE HAM clock gate while the
   k/q DMAs are in flight.
 * GpSimd is never given a data op, avoiding its ~6us one-time ucode
   IRAM load at kernel start.

The device returns unnormalized agg^T (128 rows = 2 heads x 64 dims) and
Z; the host divides, transposes and concatenates the 8 per-core slices.
"""

import ctypes
import os
import sys
import types

import ml_dtypes
import numpy as np

import concourse.bacc as bacc
import concourse.bass as bass
import concourse.tile as tile
from concourse import mybir
from concourse.vector_clock import ScopedClock

F32 = mybir.dt.float32
F16 = mybir.dt.float16
BF16 = mybir.dt.bfloat16
NPBF16 = ml_dtypes.bfloat16

B = 2
H = 8
N = 2048  # nq == nk
D = 512
HD = 64
HPC = 2  # heads per core
E = HPC * HD  # 128 output dims per core
SCALING = HD ** (-0.5)
NCORES = 8
P = 128
QC = 1024  # q super-chunk (2 per core)
NQC = N // QC
NKT = N // P  # 16 k tiles
NIT = NQC * NKT  # 32 global iterations
HF = 512  # matmul free-dim chunk (one PSUM bank)
NCC = D // P  # 4 contraction chunks for the projections

ZLAG = 10  # Z packs for iteration i are emitted at iteration i+ZLAG
AVLAG = 12  # AV pack lag (V projection borrows the agg banks early on)
MLEAD = 4  # masks 0..MLEAD-1 are DMA'd in the prologue
NWARM = 44  # PE warm-up matmuls
HQ = 256  # Z/AV pack matmul free-dim (small => less PE head-of-line blocking)
KA0 = P  # columns of KT projected before the first S matmul


# ---------------------------------------------------------------------------
# Page faults are extremely slow in this sandbox (~ms each); MAP_POPULATE
# prefaults an allocation in one syscall, ~100x faster for big arrays.
# ---------------------------------------------------------------------------
_libc = ctypes.CDLL(None, use_errno=True)
_libc.mmap.restype = ctypes.c_void_p
_libc.mmap.argtypes = [
    ctypes.c_void_p,
    ctypes.c_size_t,
    ctypes.c_int,
    ctypes.c_int,
    ctypes.c_int,
    ctypes.c_long,
]


def _alloc(shape, dtype=np.float32):
    nbytes = int(np.prod(shape)) * np.dtype(dtype).itemsize
    nbytes = (nbytes + 4095) & ~4095
    p = _libc.mmap(None, nbytes, 0x3, 0x02 | 0x20 | 0x8000, -1, 0)  # RW, PRIV|ANON|POPULATE
    if p in (None, ctypes.c_void_p(-1).value):
        return np.empty(shape, dtype)
    buf = (ctypes.c_byte * nbytes).from_address(p)
    return np.frombuffer(buf, dtype=dtype, count=int(np.prod(shape))).reshape(shape)


# ---------------------------------------------------------------------------
# Environment shim: walrus in this container rejects >1 sync wait on
# CTRL-class instructions (NoOp/Drain), but TileContext's kernel-tail drain
# carries one wait per live semaphore.  Re-emit them as individual wait_ge
# instructions (one wait each) before a bare drain.  The second
# all-engine barrier (after the semaphore clear) is dropped: nothing in
# the kernel runs after it, and the runtime joins all engine streams at
# NEFF end anyway.  Saves ~5us of butterfly-barrier ping-pong.
# ---------------------------------------------------------------------------
def _drain_and_barrier(self, tick_clock, wait_clock):
    probe = mybir.InstNoOp(
        name="wait_probe", ins=[], outs=[], engine=mybir.EngineType.SP
    )
    wait_clock.add_sem_waits(probe, ScopedClock({None: tick_clock.global_clock}))
    waits = list(probe.sync_info.on_wait) if probe.sync_info else []
    allocated = self.sems.allocated()
    by_name = {}
    for k, h in allocated.items():
        by_name[getattr(h, "name", str(k))] = h
    for w in waits:
        h = by_name.get(w.ant_name)
        assert h is not None, (w.ant_name, sorted(by_name))
        self.nc.sync.wait_ge(h, w.wait_value)
    self.nc.sync.drain()
    self.nc.all_engine_barrier(sem_only=True)
    popped = self.nc._tile_sem_poison_stack.pop()
    assert popped is self._sem_poison
    self.nc.clear_and_free_semaphores(list(allocated.values()))


def _install_shims():
    tile.TileContext._drain_and_barrier = _drain_and_barrier
    if "antenv.axon_hooks" not in sys.modules:
        try:
            from trn_agent_boot.trn_boot import _ntff_profile_via_ctypes

            mod = types.ModuleType("antenv.axon_hooks")
            hook = _ntff_profile_via_ctypes("/opt/axon/libaxon_pjrt.so")
            mod.get_axon_ntff_profile_hook = lambda: hook
            mod.set_axon_ntff_profile_hook = lambda h: None
            sys.modules["antenv.axon_hooks"] = mod
        except Exception:
            pass


# ---------------------------------------------------------------------------
# Device kernel (one Bass program, SPMD over 8 cores; shards via in_maps)
# ---------------------------------------------------------------------------
def build_nc() -> bass.Bass:
    nc = bacc.Bacc("TRN2", target_bir_lowering=False, debug=False)

    # x[p, c, n]: per-partition contiguous (8 KiB/partition) for full-rate
    # DMA.  d = c*128 + p.  qa/kb/qb/v are one half (1024 cols) each; ka is
    # split so k-tile 0 can be projected the moment 128 KiB have landed.
    qah = nc.dram_tensor("qah", [P, NCC, QC], BF16, kind="ExternalInput")
    ka1h = nc.dram_tensor("ka1h", [P, NCC, KA0], BF16, kind="ExternalInput")
    ka2h = nc.dram_tensor("ka2h", [P, NCC, QC - KA0], BF16, kind="ExternalInput")
    kbh = nc.dram_tensor("kbh", [P, NCC, QC], BF16, kind="ExternalInput")
    qbh = nc.dram_tensor("qbh", [P, NCC, QC], BF16, kind="ExternalInput")
    v0h = nc.dram_tensor("v0h", [P, NCC, QC], BF16, kind="ExternalInput")
    v1h = nc.dram_tensor("v1h", [P, NCC, QC], BF16, kind="ExternalInput")
    # weights pre-tiled on the host to the SBUF layout [p, c, e]
    wq = nc.dram_tensor("wq", [P, NCC, E], BF16, kind="ExternalInput")
    wk = nc.dram_tensor("wk", [P, NCC, E], BF16, kind="ExternalInput")
    wv = nc.dram_tensor("wv", [P, NCC, E], BF16, kind="ExternalInput")
    bqk = nc.dram_tensor("bqk", [E, 2], F32, kind="ExternalInput")  # bq|bk
    bvb = nc.dram_tensor("bvb", [P, E], F32, kind="ExternalInput")  # bv bcast
    onesd = nc.dram_tensor("onesd", [P, P], BF16, kind="ExternalInput")
    # maskt[qc, kt, k, lh, q] = mask[b, h0+lh, qc*QC+q, kt*P+k]
    #                          * diag(pearson)[b, h0+lh, kt*P+k]
    maskt = nc.dram_tensor("maskt", [NQC, NKT, P, HPC, QC], BF16, kind="ExternalInput")
    outT = nc.dram_tensor("outT", [E, N], F16, kind="ExternalOutput")
    # softmax denominators, normalization happens on the host
    zout = nc.dram_tensor("zout", [HPC, N], F16, kind="ExternalOutput")

    with tile.TileContext(nc) as tc:
        with (
            tc.tile_pool(name="consts", bufs=1) as consts,
            tc.tile_pool(name="persist", bufs=1) as persist,
            tc.tile_pool(name="ps", bufs=1, space="PSUM") as ps,
            tc.tile_pool(name="qkv", bufs=1) as qkv,
            tc.tile_pool(name="et", bufs=ZLAG + 3) as etp,
            tc.tile_pool(name="at", bufs=AVLAG + 2) as atp,
            tc.tile_pool(name="mask", bufs=MLEAD + 2) as maskp,
            tc.tile_pool(name="small", bufs=2) as smallp,
            tc.tile_pool(name="outp", bufs=2) as outp,
        ):
            def s_tile(sl, name):
                return ps.tile([P, QC], F32, tag=f"s{sl}", name=name)

            def agg_tag_tile(name, shape=(P, QC)):
                return ps.tile(list(shape), F32, tag="agg", name=name)

            def z_tag_tile(name, shape=(33, QC)):
                return ps.tile(list(shape), F32, tag="z", name=name)

            # ---- constants & input DMAs (ordering = HWDGE FIFO order) -----
            # Priority: everything the first exp needs (ones for warm-up,
            # wk/wq/bqk, qa, ka k-tile 0), then mask0 and the deferred
            # projection inputs interleaved with the early masks.
            ones = consts.tile([P, P], BF16, tag="ones")
            nc.sync.dma_start(out=ones, in_=onesd[:, :])

            qa_t = qkv.tile([P, NCC, QC], BF16, tag="qa")
            ka1_t = qkv.tile([P, NCC, KA0], BF16, tag="ka1")
            ka2_t = qkv.tile([P, NCC, QC - KA0], BF16, tag="ka2")
            kb_t = qkv.tile([P, NCC, QC], BF16, tag="kb")
            qb_t = qkv.tile([P, NCC, QC], BF16, tag="qb")
            v0_t = qkv.tile([P, NCC, QC], BF16, tag="v0")
            nc.sync.dma_start(out=qa_t, in_=qah[:, :, :])
            wq_sb = consts.tile([P, NCC, E], BF16, tag="wq")
            nc.sync.dma_start(out=wq_sb, in_=wq[:, :, :])
            wk_sb = consts.tile([P, NCC, E], BF16, tag="wk")
            nc.sync.dma_start(out=wk_sb, in_=wk[:, :, :])
            nc.sync.dma_start(out=ka1_t, in_=ka1h[:, :, :])
            bqk_sb = consts.tile([E, 2], F32, tag="bqk")
            nc.sync.dma_start(out=bqk_sb, in_=bqk[:, :])

            maskp_tiles = {}

            def emit_mask_dma(it):
                qc, kt = divmod(it, NKT)
                mt = maskp.tile([P, HPC, QC], BF16, tag="mt", name=f"mt_{it}")
                nc.sync.dma_start(out=mt, in_=maskt[qc, kt])
                return mt

            nc.sync.dma_start(out=ka2_t, in_=ka2h[:, :, :])
            maskp_tiles[0] = emit_mask_dma(0)
            nc.sync.dma_start(out=kb_t, in_=kbh[:, :, :])
            maskp_tiles[1] = emit_mask_dma(1)
            wv_sb = consts.tile([P, NCC, E], BF16, tag="wv")
            nc.sync.dma_start(out=wv_sb, in_=wv[:, :, :])
            nc.sync.dma_start(out=qb_t, in_=qbh[:, :, :])
            nc.sync.dma_start(out=v0_t, in_=v0h[:, :, :])
            bvb_sb = consts.tile([P, E], F32, tag="bvb")
            nc.sync.dma_start(out=bvb_sb, in_=bvb[:, :])

            # Preload the exp spline table during the DMAs.
            scratch = consts.tile([P, 8], F32, tag="scratch")
            nc.scalar.activation(scratch, ones[:, 0:8], mybir.ActivationFunctionType.Exp)

            # PE warm-up: a dense burst of tiny matmuls while qa/ka stream in,
            # so the HAM clock gate opens before the projections run.
            warm_ps = s_tile(0, "warm_ps")
            for i in range(NWARM):
                nc.tensor.matmul(
                    warm_ps[0:1, 0:P],
                    ones[:, 0:1],
                    ones[:, :],
                    start=True,
                    stop=True,
                    skip_group_check=True,
                )

            # ---- first-wave projections ----------------------------------
            # [e, n] = sum_c w[c, e] * xT[c, n]; bias added during the
            # PSUM->SBUF eviction on VectorE (per-partition scalar operand).
            QT_sb = persist.tile([E, N], BF16, tag="QT")  # [e, n] 2 heads x 64
            KT_sb = persist.tile([E, N], BF16, tag="KT")
            V_sb = persist.tile([P, NKT, E], BF16, tag="V")  # [k%128, kt, e]

            def emit_proj(dst, w_sb, bias_ap, src_t, jcol, pst, spans):
                """spans: list of (src_lo, src_hi, dst_col) ranges; dst_col is
                relative to jcol*QC in dst and the psum tile.  Each span's
                eviction is emitted right after its matmuls so it overlaps
                the next span's matmuls."""
                for lo, hi, dlo in spans:
                    for c in range(NCC):
                        nc.tensor.matmul(
                            pst[:, dlo : dlo + (hi - lo)],
                            w_sb[:, c, :],
                            src_t[:, c, lo:hi],
                            start=(c == 0),
                            stop=(c == NCC - 1),
                        )
                    cols = slice(jcol * QC + dlo, jcol * QC + dlo + (hi - lo))
                    nc.vector.tensor_scalar_add(
                        dst[:, cols], pst[:, dlo : dlo + (hi - lo)], bias_ap
                    )

            # Q-A: both halves (the first S matmul streams QT[:, 0:1024])
            emit_proj(QT_sb, wq_sb, bqk_sb[:, 0:1], qa_t, 0, s_tile(1, "ps_qA"),
                      [(0, HF, 0), (HF, QC, HF)])
            # K-A k-tile 0 only: unblocks S^T(0) after ~128 KiB of ka
            emit_proj(KT_sb, wk_sb, bqk_sb[:, 1:2], ka1_t, 0, s_tile(0, "ps_kA0"),
                      [(0, KA0, 0)])
            # v1 reuses the qa staging tile (free once Q-A is projected);
            # emitted after the Q-A matmuls so the WAR dep is tracked.
            v1_t = qkv.tile([P, NCC, QC], BF16, tag="qa", name="v1_t")
            nc.sync.dma_start(out=v1_t, in_=v1h[:, :, :])
            for pit in range(2, MLEAD):
                maskp_tiles[pit] = emit_mask_dma(pit)

            def emit_v_chunk(sub, pst):
                """V natural layout: [n, e] = sum_c vT[c, n] * w[c, e].
                One sub-chunk = 4 k-tiles into half of a [128,1024] psum."""
                vt = v0_t if sub < 2 else v1_t
                for t4 in range(4):
                    t = sub * 4 + t4
                    col = (sub % 2) * 4 + t4
                    tl = (t % 8) * P
                    for c in range(NCC):
                        nc.tensor.matmul(
                            pst[:, col * E : (col + 1) * E],
                            vt[:, c, tl : tl + P],
                            wv_sb[:, c, :],
                            start=(c == 0),
                            stop=(c == NCC - 1),
                        )
                for t4 in range(4):
                    t = sub * 4 + t4
                    col = (sub % 2) * 4 + t4
                    nc.vector.tensor_add(
                        V_sb[:, t, :], pst[:, col * E : (col + 1) * E], bvb_sb
                    )

            # ---- attention: 32 software-pipelined iterations ---------------
            def emit_s(it, lh, s_ps):
                qc, kt = divmod(it, NKT)
                kcols = slice(kt * P, (kt + 1) * P)
                hsl = slice(lh * HD, (lh + 1) * HD)
                for half in range(QC // HF):
                    rcols = slice(qc * QC + half * HF, qc * QC + (half + 1) * HF)
                    nc.tensor.matmul(
                        s_ps[:, half * HF : (half + 1) * HF],
                        KT_sb[hsl, kcols],
                        QT_sb[hsl, rcols],
                        start=True,
                        stop=True,
                        tile_position=(lh * HD, 0),
                    )

            ets = {}
            ats = {}
            aggs = {}
            zpss = {}

            def emit_z_pack(it):
                qc, kt = divmod(it, NKT)
                if kt == 0:
                    zpss[qc] = z_tag_tile(f"z{qc}")
                zps, et = zpss[qc], ets.pop(it)
                for half in range(QC // HQ):
                    hcols = slice(half * HQ, (half + 1) * HQ)
                    # start=True clears the has_written bits of the WHOLE
                    # psum bank (for the written partitions), so only the
                    # first sub-bank group may carry it; the second group's
                    # kt==0 matmul overwrites thanks to the cleared bits.
                    first = kt == 0 and (half * HQ) % HF == 0
                    for lh in range(HPC):
                        nc.tensor.matmul(
                            zps[lh * 32 : lh * 32 + 1, hcols],
                            ones[:, 0:1],
                            et[:, lh, hcols],
                            start=first,
                            stop=(kt == NKT - 1),
                            tile_position=(0, lh * 32),
                            skip_group_check=True,
                        )

            def emit_av_pack(it):
                qc, kt = divmod(it, NKT)
                if kt == 0:
                    aggs[qc] = agg_tag_tile(f"agg{qc}")
                agg, at = aggs[qc], ats.pop(it)
                for half in range(QC // HQ):
                    hcols = slice(half * HQ, (half + 1) * HQ)
                    first = kt == 0 and (half * HQ) % HF == 0
                    for lh in range(HPC):
                        esl = slice(lh * HD, (lh + 1) * HD)
                        nc.tensor.matmul(
                            agg[esl, hcols],
                            V_sb[:, kt, esl],
                            at[:, lh, hcols],
                            start=first,
                            stop=(kt == NKT - 1),
                            tile_position=(0, lh * HD),
                            skip_group_check=True,
                        )

            def emit_epilogue(qc):
                qcols = slice(qc * QC, (qc + 1) * QC)
                zps, agg = zpss.pop(qc), aggs.pop(qc)
                # half-split the agg eviction so each PSUM->SBUF copy's
                # DMA overlaps the next copy; z last (tiny, never gating)
                osb = outp.tile([P, QC], F16, tag="osb", name=f"osb_{qc}")
                for half in range(2):
                    hsl = slice(half * HF, (half + 1) * HF)
                    nc.vector.tensor_copy(osb[:, hsl], agg[:, hsl])
                    nc.sync.dma_start(
                        out=outT[:, qc * QC + half * HF : qc * QC + (half + 1) * HF],
                        in_=osb[:, hsl],
                    )
                zsb = smallp.tile([33, QC], F16, tag="zsb", name=f"zsb{qc}")
                nc.vector.tensor_copy(zsb, zps)
                for lh in range(HPC):
                    nc.sync.dma_start(
                        out=zout[lh, qcols], in_=zsb[lh * 32 : lh * 32 + 1, :]
                    )

            s_cur = [s_tile(lh, f"s_0_{lh}") for lh in range(HPC)]
            for lh in range(HPC):
                emit_s(0, lh, s_cur[lh])
            # K-A remainder, first span (KT cols 128:512) right behind
            # S^T(0) in the PE FIFO: S^T(1..3) read these columns, so the
            # writes must precede them.  The second span (cols 512:1024,
            # first needed by S^T(4)) is wedged at iteration 1 so S^T(1)
            # doesn't queue behind it.
            emit_proj(KT_sb, wk_sb, bqk_sb[:, 1:2], ka2_t, 0,
                      agg_tag_tile("ps_kA2a"),
                      [(0, HF - KA0, KA0)])

            z_next = 0
            av_next = 0
            epi_done = 0
            v_ps = [None, None]  # borrowed psum tiles for the V sub-chunks

            def try_epi():
                nonlocal epi_done
                while (
                    epi_done < NQC
                    and z_next > epi_done * NKT + NKT - 1
                    and av_next > epi_done * NKT + NKT - 1
                ):
                    emit_epilogue(epi_done)
                    epi_done += 1

            WEDGE_ITS = {4, 7, 8, 9, 10, 11}

            def drain_packs(it):
                """Emit pending Z/AV packs under a per-iteration PE budget.

                The packs sit in the PE FIFO between S^T(it+1) and
                S^T(it+2); the exp cadence (2.24us/iter) minus the S
                matmuls (0.86us) leaves room for ~3 packs of 0.43us each.
                Iterations carrying a projection wedge get 2.  Z packs only
                need et and the z banks (free after the Q-B wedge at it=8),
                so they start at it=9; AV packs need masks and V_sb, so
                they start at it=13.  23 iterations x ~3 packs >= 64 packs
                drains everything by the last iteration.  A pack may not
                enter a new qc until that qc's predecessor epilogue is out
                (its first matmul re-claims the banks the epilogue eviction
                reads)."""
                nonlocal z_next, av_next
                budget = 2 if (it in WEDGE_ITS or it % 4 == 2) else 3
                if it >= 30 or it >= NIT:
                    budget = 6
                zlim = it - 1 if it >= ZLAG - 1 else -1
                alim = it - 3 if it >= AVLAG else -1
                if it >= 29:
                    alim = it
                while budget > 0:
                    jz, ja = z_next, av_next
                    z_ok = (
                        jz < NIT
                        and jz <= zlim
                        and not (jz % NKT == 0 and jz > 0 and epi_done < jz // NKT)
                    )
                    a_ok = (
                        ja < NIT
                        and ja <= alim
                        and not (ja % NKT == 0 and ja > 0 and epi_done < ja // NKT)
                    )
                    if not z_ok and not a_ok:
                        break
                    # alternate, preferring whichever stream is furthest behind
                    if z_ok and (not a_ok or jz - ja <= 4):
                        emit_z_pack(jz)
                        z_next += 1
                    else:
                        emit_av_pack(ja)
                        av_next += 1
                    budget -= 1
                    try_epi()

            for it in range(NIT):
                last = it == NIT - 1
                if it + MLEAD < NIT:
                    maskp_tiles[it + MLEAD] = emit_mask_dma(it + MLEAD)

                # deferred projections, wedged at the TOP of the body so
                # their PSUM->SBUF evictions sit in the DVE FIFO ahead of
                # this iteration's (possibly mask-gated) A^T multiplies.
                # Each wedge's inputs are in SBUF just before the PE's FIFO
                # reaches it, so it never blocks the S matmuls behind it.
                if it == 1:
                    emit_proj(KT_sb, wk_sb, bqk_sb[:, 1:2], ka2_t, 0,
                              agg_tag_tile("ps_kA2b"),
                              [(HF - KA0, QC - KA0, HF)])
                elif it == 4:
                    emit_proj(KT_sb, wk_sb, bqk_sb[:, 1:2], kb_t, 1,
                              z_tag_tile("ps_kB", shape=(P, QC)),
                              [(0, HF, 0), (HF, QC, HF)])
                elif it == 8:
                    emit_proj(QT_sb, wq_sb, bqk_sb[:, 0:1], qb_t, 1,
                              z_tag_tile("ps_qB", shape=(P, QC)),
                              [(0, HF, 0), (HF, QC, HF)])
                elif it == 7:
                    v_ps[0] = agg_tag_tile("ps_vA")
                    emit_v_chunk(0, v_ps[0])
                elif it == 9:
                    emit_v_chunk(1, v_ps[0])
                elif it == 10:
                    v_ps[1] = agg_tag_tile("ps_vB")
                    emit_v_chunk(2, v_ps[1])
                elif it == 11:
                    emit_v_chunk(3, v_ps[1])

                mt = maskp_tiles.pop(it)
                et = etp.tile([P, HPC, QC], BF16, tag="et", name=f"et_{it}")
                at = atp.tile([P, HPC, QC], BF16, tag="at", name=f"at_{it}")
                ets[it], ats[it] = et, at
                s_nxt = (
                    [s_tile(lh, f"s_{it + 1}_{lh}") for lh in range(HPC)]
                    if not last
                    else None
                )
                for lh in range(HPC):
                    nc.scalar.activation(
                        et[:, lh, :],
                        s_cur[lh],
                        mybir.ActivationFunctionType.Exp,
                        scale=SCALING,
                    )
                    nc.vector.tensor_mul(at[:, lh, :], et[:, lh, :], mt[:, lh, :])
                    # S^T for the next iteration reuses this head's PSUM
                    # banks; emit right after the exp that frees them.
                    if not last:
                        emit_s(it + 1, lh, s_nxt[lh])
                s_cur = s_nxt
                drain_packs(it)

            it = NIT
            while z_next < NIT or av_next < NIT:
                drain_packs(it)
                it += 1

    nc.compile()
    return nc


# ---------------------------------------------------------------------------
# Host side
# ---------------------------------------------------------------------------
def _prep_in_maps(q, k, v, mask_head, pearson_matrix, Wq, bq, Wk, bk, Wv, bv):
    f = np.float32
    q = np.asarray(q, f)
    k = np.asarray(k, f)
    v = np.asarray(v, f)
    mask_head = np.asarray(mask_head, f)
    Wq = np.asarray(Wq, f)
    Wk = np.asarray(Wk, f)
    Wv = np.asarray(Wv, f)
    bq = np.asarray(bq, f).reshape(D)
    bk = np.asarray(bk, f).reshape(D)
    bv = np.asarray(bv, f).reshape(D)

    # Only the diagonal of pearson is used by the computation.
    pm = np.asarray(pearson_matrix)
    diag = np.ascontiguousarray(np.diagonal(pm, axis1=-2, axis2=-1)).astype(f)

    def _ptile(x, lo, hi):
        """x [n, d] -> [p, c, hi-lo] bf16 with d = c*128 + p, cols lo:hi of
        the transposed matrix; per-partition contiguous."""
        dst = _alloc((P, NCC, hi - lo), NPBF16)
        xT = x.T.reshape(NCC, P, x.shape[0])  # [c, p, n]
        np.copyto(dst, xT[:, :, lo:hi].transpose(1, 0, 2))
        return dst

    qT = [[_ptile(q[b], 0, QC), _ptile(q[b], QC, N)] for b in range(B)]
    kT = [
        [_ptile(k[b], 0, KA0), _ptile(k[b], KA0, QC), _ptile(k[b], QC, N)]
        for b in range(B)
    ]
    vT = [[_ptile(v[b], 0, QC), _ptile(v[b], QC, N)] for b in range(B)]
    onesd = np.ones((P, P), NPBF16)

    def wtile(W, esl):
        # [D, E] -> [P, NCC, E] with d = c*P + p
        wT = np.ascontiguousarray(W[esl, :].T.astype(NPBF16))
        return np.ascontiguousarray(wT.reshape(NCC, P, E).transpose(1, 0, 2))

    # Per-(b,h) mask, transposed to [k, q], diag-folded, tiled to the exact
    # per-iteration consumption order: [qc, kt, k, lh, q].
    maskt_all = _alloc((B, H // HPC, NQC, NKT, P, HPC, QC), NPBF16)
    for b in range(B):
        for h in range(H):
            md = mask_head[b, h].T * diag[b, h][:, None]  # [k, q] f32
            tiled = md.reshape(NKT, P, NQC, QC).transpose(2, 0, 1, 3)
            maskt_all[b, h // HPC, :, :, :, h % HPC, :] = tiled

    in_maps = []
    for c in range(NCORES):
        b = c // (NCORES // B)
        h0 = HPC * (c % (NCORES // B))
        esl = slice(h0 * HD, (h0 + HPC) * HD)
        bqk_h = np.ascontiguousarray(
            np.stack([bq[esl], bk[esl]], axis=1).astype(f)
        )
        bvb_h = np.ascontiguousarray(
            np.broadcast_to(bv[esl][None, :], (P, E)).astype(f)
        )
        in_maps.append(
            {
                "qah": qT[b][0],
                "qbh": qT[b][1],
                "ka1h": kT[b][0],
                "ka2h": kT[b][1],
                "kbh": kT[b][2],
                "v0h": vT[b][0],
                "v1h": vT[b][1],
                "wq": wtile(Wq, esl),
                "wk": wtile(Wk, esl),
                "wv": wtile(Wv, esl),
                "bqk": bqk_h,
                "bvb": bvb_h,
                "onesd": onesd,
                "maskt": maskt_all[b, h0 // HPC],
            }
        )
    return in_maps


_NC_CACHE = None
LAST_RESULT = None  # BassKernelResults of the most recent run (for profiling)


def kernel(**inputs) -> np.ndarray:
    global _NC_CACHE, LAST_RESULT
    _install_shims()
    from concourse.bass_utils import run_bass_kernel_spmd

    if _NC_CACHE is None:
        _NC_CACHE = build_nc()
    nc = _NC_CACHE

    in_maps = _prep_in_maps(**inputs)

    trace = bool(int(os.environ.get("KERNEL_TRACE", "0")))
    kwargs = {}
    if trace:
        kwargs["trace"] = True
        tmpdir = os.environ.get("KERNEL_TRACE_DIR")
        if tmpdir:
            kwargs["tmpdir"] = tmpdir
    res = run_bass_kernel_spmd(nc, in_maps, list(range(NCORES)), **kwargs)
    LAST_RESULT = res

    out = _alloc((B, N, D), np.float32)
    for c in range(NCORES):
        b = c // (NCORES // B)
        h0 = HPC * (c % (NCORES // B))
        aggT = np.asarray(res.results[c]["outT"], np.float32)  # (E, N)
        z = np.asarray(res.results[c]["zout"], np.float32)  # (HPC, N)
        out[b, :, h0 * HD : (h0 + HPC) * HD] = (
            aggT / np.repeat(z, HD, axis=0)
        ).T
    return out


# revision 34
# speedup vs baseline: 1.0225x; 1.0225x over previous
"""MultiHeadCrossAttention kernel for 8 Trainium2 NeuronCores.

Reference computation (b=2, nq=nk=2048, d_model=512, h=8, hd=64):
    Q = split_heads(q @ Wq.T + bq); K, V likewise
    S = Q K^T * hd^-0.5 ; A = softmax(S, -1) * mask_head * diag(pearson)
    out = merge_heads(A @ V)

Sharding: 16 (batch, head) pairs -> 2 heads of one batch per core.

Only the *diagonal* of pearson_matrix is used, so it is extracted on the
host and folded into the mask.  The mask is transposed, diag-folded,
tiled to the exact per-iteration consumption order and cast to bf16 on
the host, so every device-side mask DMA is one contiguous 512 KiB read.

All matmul operands are bf16 (PE streams 1 column/cycle for bf16; fp16
runs at half that rate).  PSUM accumulation stays fp32.  bf16's 8
mantissa bits keep the rel-err ~4e-3, under the 2e-2 gate.

Device layout per core ("k on partitions, q on free axis"):
    S^T[k,q]   = sum_d K^T[d,k] Q^T[d,q]       (TensorE, d=64 contraction,
                                                2 heads run concurrently in
                                                disjoint row halves)
    E^T        = exp(SCALING * S^T)            (ScalarE, PSUM->SBUF bf16)
    Z[q]      += ones^T @ E^T                  (TensorE, PSUM-accumulated,
                                                heads packed in col groups
                                                0/32 -> run concurrently)
    A^T        = E^T * maskT_folded            (VectorE, bf16 2x mode)
    agg^T[e,q]+= sum_k V[k,e] A^T[k,q]         (TensorE, PSUM-accumulated,
                                                heads col-packed 0/64)

The scalar-engine exp stream (~1.12us per [128,1024] tile, 64 tiles) is
the critical path; everything else is scheduled around keeping it dense:

 * k/q/v DRAM staging is per-partition contiguous so DMAs run at full
   rate; the DMA queue order puts qa + the first k-tile of ka ahead of
   everything else so the first S^T matmul (and exp) start ~15us in.
 * The K-A projection is split: k-tile 0 only (gating exp #0), then the
   rest wedged into iteration 1.
 * Z/AV matmul packs are emitted several iterations late (ZLAG/AVLAG) so
   the PE never blocks the S->exp chain on mask/projection stragglers.
 * The K-B/Q-B/V projections are wedged into early iterations, borrowing
   the agg/z PSUM tags before the (lagged) first Z/AV accumulations
   claim them.
 * A burst of tiny warm-up matmuls opens the PE HAM clock gate while the
   k/q DMAs are in flight.
 * GpSimd is never given a data op, avoiding its ~6us one-time ucode
   IRAM load at kernel start.

The device returns unnormalized agg^T (128 rows = 2 heads x 64 dims) and
Z; the host divides, transposes and concatenates the 8 per-core slices.
"""

import ctypes
import os
import sys
import types

import ml_dtypes
import numpy as np

import concourse.bacc as bacc
import concourse.bass as bass
import concourse.tile as tile
from concourse import mybir
from concourse.vector_clock import ScopedClock

F32 = mybir.dt.float32
F16 = mybir.dt.float16
BF16 = mybir.dt.bfloat16
NPBF16 = ml_dtypes.bfloat16

B = 2
H = 8
N = 2048  # nq == nk
D = 512
HD = 64
HPC = 2  # heads per core
E = HPC * HD  # 128 output dims per core
SCALING = HD ** (-0.5)
NCORES = 8
P = 128
QC = 1024  # q super-chunk (2 per core)
NQC = N // QC
NKT = N // P  # 16 k tiles
NIT = NQC * NKT  # 32 global iterations
HF = 512  # matmul free-dim chunk (one PSUM bank)
NCC = D // P  # 4 contraction chunks for the projections

ZLAG = 10  # Z packs for iteration i are emitted at iteration i+ZLAG
AVLAG = 13  # AV pack lag (V projection borrows the agg banks early on)
MLEAD = 4  # masks 0..MLEAD-1 are DMA'd in the prologue
NWARM = 44  # PE warm-up matmuls
HQ = 256  # Z/AV pack matmul free-dim (small => less PE head-of-line blocking)
KA0 = P  # columns of KT projected before the first S matmul


# ---------------------------------------------------------------------------
# Page faults are extremely slow in this sandbox (~ms each); MAP_POPULATE
# prefaults an allocation in one syscall, ~100x faster for big arrays.
# ---------------------------------------------------------------------------
_libc = ctypes.CDLL(None, use_errno=True)
_libc.mmap.restype = ctypes.c_void_p
_libc.mmap.argtypes = [
    ctypes.c_void_p,
    ctypes.c_size_t,
    ctypes.c_int,
    ctypes.c_int,
    ctypes.c_int,
    ctypes.c_long,
]


def _alloc(shape, dtype=np.float32):
    nbytes = int(np.prod(shape)) * np.dtype(dtype).itemsize
    nbytes = (nbytes + 4095) & ~4095
    p = _libc.mmap(None, nbytes, 0x3, 0x02 | 0x20 | 0x8000, -1, 0)  # RW, PRIV|ANON|POPULATE
    if p in (None, ctypes.c_void_p(-1).value):
        return np.empty(shape, dtype)
    buf = (ctypes.c_byte * nbytes).from_address(p)
    return np.frombuffer(buf, dtype=dtype, count=int(np.prod(shape))).reshape(shape)


# ---------------------------------------------------------------------------
# Environment shim: walrus in this container rejects >1 sync wait on
# CTRL-class instructions (NoOp/Drain), but TileContext's kernel-tail drain
# carries one wait per live semaphore.  Re-emit them as individual wait_ge
# instructions (one wait each) before a bare drain.  The second
# all-engine barrier (after the semaphore clear) is dropped: nothing in
# the kernel runs after it, and the runtime joins all engine streams at
# NEFF end anyway.  Saves ~5us of butterfly-barrier ping-pong.
# ---------------------------------------------------------------------------
def _drain_and_barrier(self, tick_clock, wait_clock):
    probe = mybir.InstNoOp(
        name="wait_probe", ins=[], outs=[], engine=mybir.EngineType.SP
    )
    wait_clock.add_sem_waits(probe, ScopedClock({None: tick_clock.global_clock}))
    waits = list(probe.sync_info.on_wait) if probe.sync_info else []
    allocated = self.sems.allocated()
    by_name = {}
    for k, h in allocated.items():
        by_name[getattr(h, "name", str(k))] = h
    for w in waits:
        h = by_name.get(w.ant_name)
        assert h is not None, (w.ant_name, sorted(by_name))
        self.nc.sync.wait_ge(h, w.wait_value)
    self.nc.sync.drain()
    self.nc.all_engine_barrier(sem_only=True)
    popped = self.nc._tile_sem_poison_stack.pop()
    assert popped is self._sem_poison
    self.nc.clear_and_free_semaphores(list(allocated.values()))


def _install_shims():
    tile.TileContext._drain_and_barrier = _drain_and_barrier
    if "antenv.axon_hooks" not in sys.modules:
        try:
            from trn_agent_boot.trn_boot import _ntff_profile_via_ctypes

            mod = types.ModuleType("antenv.axon_hooks")
            hook = _ntff_profile_via_ctypes("/opt/axon/libaxon_pjrt.so")
            mod.get_axon_ntff_profile_hook = lambda: hook
            mod.set_axon_ntff_profile_hook = lambda h: None
            sys.modules["antenv.axon_hooks"] = mod
        except Exception:
            pass


# ---------------------------------------------------------------------------
# Device kernel (one Bass program, SPMD over 8 cores; shards via in_maps)
# ---------------------------------------------------------------------------
def build_nc() -> bass.Bass:
    nc = bacc.Bacc("TRN2", target_bir_lowering=False, debug=False)

    # x[p, c, n]: per-partition contiguous (8 KiB/partition) for full-rate
    # DMA.  d = c*128 + p.  qa/kb/qb/v are one half (1024 cols) each; ka is
    # split so k-tile 0 can be projected the moment 128 KiB have landed.
    qah = nc.dram_tensor("qah", [P, NCC, QC], BF16, kind="ExternalInput")
    ka1h = nc.dram_tensor("ka1h", [P, NCC, KA0], BF16, kind="ExternalInput")
    ka2h = nc.dram_tensor("ka2h", [P, NCC, QC - KA0], BF16, kind="ExternalInput")
    kbh = nc.dram_tensor("kbh", [P, NCC, QC], BF16, kind="ExternalInput")
    qbh = nc.dram_tensor("qbh", [P, NCC, QC], BF16, kind="ExternalInput")
    v0h = nc.dram_tensor("v0h", [P, NCC, QC], BF16, kind="ExternalInput")
    v1h = nc.dram_tensor("v1h", [P, NCC, QC], BF16, kind="ExternalInput")
    # weights pre-tiled on the host to the SBUF layout [p, c, e]
    wq = nc.dram_tensor("wq", [P, NCC, E], BF16, kind="ExternalInput")
    wk = nc.dram_tensor("wk", [P, NCC, E], BF16, kind="ExternalInput")
    wv = nc.dram_tensor("wv", [P, NCC, E], BF16, kind="ExternalInput")
    bqk = nc.dram_tensor("bqk", [E, 2], F32, kind="ExternalInput")  # bq|bk
    bvb = nc.dram_tensor("bvb", [P, E], F32, kind="ExternalInput")  # bv bcast
    onesd = nc.dram_tensor("onesd", [P, P], BF16, kind="ExternalInput")
    # maskt[qc, kt, k, lh, q] = mask[b, h0+lh, qc*QC+q, kt*P+k]
    #                          * diag(pearson)[b, h0+lh, kt*P+k]
    maskt = nc.dram_tensor("maskt", [NQC, NKT, P, HPC, QC], BF16, kind="ExternalInput")
    outT = nc.dram_tensor("outT", [E, N], F16, kind="ExternalOutput")
    # softmax denominators, normalization happens on the host
    zout = nc.dram_tensor("zout", [HPC, N], F16, kind="ExternalOutput")

    with tile.TileContext(nc) as tc:
        with (
            tc.tile_pool(name="consts", bufs=1) as consts,
            tc.tile_pool(name="persist", bufs=1) as persist,
            tc.tile_pool(name="ps", bufs=1, space="PSUM") as ps,
            tc.tile_pool(name="qkv", bufs=1) as qkv,
            tc.tile_pool(name="et", bufs=ZLAG + 3) as etp,
            tc.tile_pool(name="at", bufs=AVLAG + 2) as atp,
            tc.tile_pool(name="mask", bufs=MLEAD + 2) as maskp,
            tc.tile_pool(name="small", bufs=2) as smallp,
            tc.tile_pool(name="outp", bufs=2) as outp,
        ):
            def s_tile(sl, name):
                return ps.tile([P, QC], F32, tag=f"s{sl}", name=name)

            def agg_tag_tile(name, shape=(P, QC)):
                return ps.tile(list(shape), F32, tag="agg", name=name)

            def z_tag_tile(name, shape=(33, QC)):
                return ps.tile(list(shape), F32, tag="z", name=name)

            # ---- constants & input DMAs (ordering = HWDGE FIFO order) -----
            # Priority: everything the first exp needs (ones for warm-up,
            # wk/wq/bqk, qa, ka k-tile 0), then mask0 and the deferred
            # projection inputs interleaved with the early masks.
            ones = consts.tile([P, P], BF16, tag="ones")
            nc.sync.dma_start(out=ones, in_=onesd[:, :])

            qa_t = qkv.tile([P, NCC, QC], BF16, tag="qa")
            ka1_t = qkv.tile([P, NCC, KA0], BF16, tag="ka1")
            ka2_t = qkv.tile([P, NCC, QC - KA0], BF16, tag="ka2")
            kb_t = qkv.tile([P, NCC, QC], BF16, tag="kb")
            qb_t = qkv.tile([P, NCC, QC], BF16, tag="qb")
            v0_t = qkv.tile([P, NCC, QC], BF16, tag="v0")
            nc.sync.dma_start(out=qa_t, in_=qah[:, :, :])
            wq_sb = consts.tile([P, NCC, E], BF16, tag="wq")
            nc.sync.dma_start(out=wq_sb, in_=wq[:, :, :])
            wk_sb = consts.tile([P, NCC, E], BF16, tag="wk")
            nc.sync.dma_start(out=wk_sb, in_=wk[:, :, :])
            nc.sync.dma_start(out=ka1_t, in_=ka1h[:, :, :])
            bqk_sb = consts.tile([E, 2], F32, tag="bqk")
            nc.sync.dma_start(out=bqk_sb, in_=bqk[:, :])

            maskp_tiles = {}

            def emit_mask_dma(it):
                qc, kt = divmod(it, NKT)
                mt = maskp.tile([P, HPC, QC], BF16, tag="mt", name=f"mt_{it}")
                nc.sync.dma_start(out=mt, in_=maskt[qc, kt])
                return mt

            nc.sync.dma_start(out=ka2_t, in_=ka2h[:, :, :])
            maskp_tiles[0] = emit_mask_dma(0)
            nc.sync.dma_start(out=kb_t, in_=kbh[:, :, :])
            maskp_tiles[1] = emit_mask_dma(1)
            wv_sb = consts.tile([P, NCC, E], BF16, tag="wv")
            nc.sync.dma_start(out=wv_sb, in_=wv[:, :, :])
            nc.sync.dma_start(out=qb_t, in_=qbh[:, :, :])
            nc.sync.dma_start(out=v0_t, in_=v0h[:, :, :])
            bvb_sb = consts.tile([P, E], F32, tag="bvb")
            nc.sync.dma_start(out=bvb_sb, in_=bvb[:, :])

            # Preload the exp spline table during the DMAs.
            scratch = consts.tile([P, 8], F32, tag="scratch")
            nc.scalar.activation(scratch, ones[:, 0:8], mybir.ActivationFunctionType.Exp)

            # PE warm-up: a dense burst of tiny matmuls while qa/ka stream in,
            # so the HAM clock gate opens before the projections run.
            warm_ps = s_tile(0, "warm_ps")
            for i in range(NWARM):
                nc.tensor.matmul(
                    warm_ps[0:1, 0:P],
                    ones[:, 0:1],
                    ones[:, :],
                    start=True,
                    stop=True,
                    skip_group_check=True,
                )

            # ---- first-wave projections ----------------------------------
            # [e, n] = sum_c w[c, e] * xT[c, n]; bias added during the
            # PSUM->SBUF eviction on VectorE (per-partition scalar operand).
            QT_sb = persist.tile([E, N], BF16, tag="QT")  # [e, n] 2 heads x 64
            KT_sb = persist.tile([E, N], BF16, tag="KT")
            V_sb = persist.tile([P, NKT, E], BF16, tag="V")  # [k%128, kt, e]

            def emit_proj(dst, w_sb, bias_ap, src_t, jcol, pst, spans):
                """spans: list of (src_lo, src_hi, dst_col) ranges; dst_col is
                relative to jcol*QC in dst and the psum tile.  Each span's
                eviction is emitted right after its matmuls so it overlaps
                the next span's matmuls."""
                for lo, hi, dlo in spans:
                    for c in range(NCC):
                        nc.tensor.matmul(
                            pst[:, dlo : dlo + (hi - lo)],
                            w_sb[:, c, :],
                            src_t[:, c, lo:hi],
                            start=(c == 0),
                            stop=(c == NCC - 1),
                        )
                    cols = slice(jcol * QC + dlo, jcol * QC + dlo + (hi - lo))
                    nc.vector.tensor_scalar_add(
                        dst[:, cols], pst[:, dlo : dlo + (hi - lo)], bias_ap
                    )

            # Q-A: both halves (the first S matmul streams QT[:, 0:1024])
            emit_proj(QT_sb, wq_sb, bqk_sb[:, 0:1], qa_t, 0, s_tile(1, "ps_qA"),
                      [(0, HF, 0), (HF, QC, HF)])
            # K-A k-tile 0 only: unblocks S^T(0) after ~128 KiB of ka
            emit_proj(KT_sb, wk_sb, bqk_sb[:, 1:2], ka1_t, 0, s_tile(0, "ps_kA0"),
                      [(0, KA0, 0)])
            # v1 reuses the qa staging tile (free once Q-A is projected);
            # emitted after the Q-A matmuls so the WAR dep is tracked.
            v1_t = qkv.tile([P, NCC, QC], BF16, tag="qa", name="v1_t")
            nc.sync.dma_start(out=v1_t, in_=v1h[:, :, :])
            for pit in range(2, MLEAD):
                maskp_tiles[pit] = emit_mask_dma(pit)

            def emit_v_chunk(sub, pst):
                """V natural layout: [n, e] = sum_c vT[c, n] * w[c, e].
                One sub-chunk = 4 k-tiles into half of a [128,1024] psum."""
                vt = v0_t if sub < 2 else v1_t
                for t4 in range(4):
                    t = sub * 4 + t4
                    col = (sub % 2) * 4 + t4
                    tl = (t % 8) * P
                    for c in range(NCC):
                        nc.tensor.matmul(
                            pst[:, col * E : (col + 1) * E],
                            vt[:, c, tl : tl + P],
                            wv_sb[:, c, :],
                            start=(c == 0),
                            stop=(c == NCC - 1),
                        )
                for t4 in range(4):
                    t = sub * 4 + t4
                    col = (sub % 2) * 4 + t4
                    nc.vector.tensor_add(
                        V_sb[:, t, :], pst[:, col * E : (col + 1) * E], bvb_sb
                    )

            # ---- attention: 32 software-pipelined iterations ---------------
            def emit_s(it, lh, s_ps):
                qc, kt = divmod(it, NKT)
                kcols = slice(kt * P, (kt + 1) * P)
                hsl = slice(lh * HD, (lh + 1) * HD)
                for half in range(QC // HF):
                    rcols = slice(qc * QC + half * HF, qc * QC + (half + 1) * HF)
                    nc.tensor.matmul(
                        s_ps[:, half * HF : (half + 1) * HF],
                        KT_sb[hsl, kcols],
                        QT_sb[hsl, rcols],
                        start=True,
                        stop=True,
                        tile_position=(lh * HD, 0),
                    )

            ets = {}
            ats = {}
            aggs = {}
            zpss = {}

            def emit_z_pack(it):
                qc, kt = divmod(it, NKT)
                if kt == 0:
                    zpss[qc] = z_tag_tile(f"z{qc}")
                zps, et = zpss[qc], ets.pop(it)
                for half in range(QC // HQ):
                    hcols = slice(half * HQ, (half + 1) * HQ)
                    # start=True clears the has_written bits of the WHOLE
                    # psum bank (for the written partitions), so only the
                    # first sub-bank group may carry it; the second group's
                    # kt==0 matmul overwrites thanks to the cleared bits.
                    first = kt == 0 and (half * HQ) % HF == 0
                    for lh in range(HPC):
                        nc.tensor.matmul(
                            zps[lh * 32 : lh * 32 + 1, hcols],
                            ones[:, 0:1],
                            et[:, lh, hcols],
                            start=first,
                            stop=(kt == NKT - 1),
                            tile_position=(0, lh * 32),
                            skip_group_check=True,
                        )

            def emit_av_pack(it):
                qc, kt = divmod(it, NKT)
                if kt == 0:
                    aggs[qc] = agg_tag_tile(f"agg{qc}")
                agg, at = aggs[qc], ats.pop(it)
                for half in range(QC // HQ):
                    hcols = slice(half * HQ, (half + 1) * HQ)
                    first = kt == 0 and (half * HQ) % HF == 0
                    for lh in range(HPC):
                        esl = slice(lh * HD, (lh + 1) * HD)
                        nc.tensor.matmul(
                            agg[esl, hcols],
                            V_sb[:, kt, esl],
                            at[:, lh, hcols],
                            start=first,
                            stop=(kt == NKT - 1),
                            tile_position=(0, lh * HD),
                            skip_group_check=True,
                        )

            def emit_epilogue(qc):
                qcols = slice(qc * QC, (qc + 1) * QC)
                zps, agg = zpss.pop(qc), aggs.pop(qc)
                # half-split the agg eviction so each PSUM->SBUF copy's
                # DMA overlaps the next copy; z last (tiny, never gating)
                osb = outp.tile([P, QC], F16, tag="osb", name=f"osb_{qc}")
                for half in range(2):
                    hsl = slice(half * HF, (half + 1) * HF)
                    nc.vector.tensor_copy(osb[:, hsl], agg[:, hsl])
                    nc.sync.dma_start(
                        out=outT[:, qc * QC + half * HF : qc * QC + (half + 1) * HF],
                        in_=osb[:, hsl],
                    )
                zsb = smallp.tile([33, QC], F16, tag="zsb", name=f"zsb{qc}")
                nc.vector.tensor_copy(zsb, zps)
                for lh in range(HPC):
                    nc.sync.dma_start(
                        out=zout[lh, qcols], in_=zsb[lh * 32 : lh * 32 + 1, :]
                    )

            s_cur = [s_tile(lh, f"s_0_{lh}") for lh in range(HPC)]
            for lh in range(HPC):
                emit_s(0, lh, s_cur[lh])
            # K-A remainder, first span (KT cols 128:512) right behind
            # S^T(0) in the PE FIFO: S^T(1..3) read these columns, so the
            # writes must precede them.  The second span (cols 512:1024,
            # first needed by S^T(4)) is wedged at iteration 1 so S^T(1)
            # doesn't queue behind it.
            emit_proj(KT_sb, wk_sb, bqk_sb[:, 1:2], ka2_t, 0,
                      agg_tag_tile("ps_kA2a"),
                      [(0, HF - KA0, KA0)])

            z_next = 0
            av_next = 0
            epi_done = 0
            v_ps = [None, None]  # borrowed psum tiles for the V sub-chunks

            def try_epi():
                nonlocal epi_done
                while (
                    epi_done < NQC
                    and z_next > epi_done * NKT + NKT - 1
                    and av_next > epi_done * NKT + NKT - 1
                ):
                    emit_epilogue(epi_done)
                    epi_done += 1

            WEDGE_ITS = {4, 8, 9, 10, 11, 12}

            def drain_packs(it):
                """Emit pending Z/AV packs under a per-iteration PE budget.

                The packs sit in the PE FIFO between S^T(it+1) and
                S^T(it+2); the exp cadence (2.24us/iter) minus the S
                matmuls (0.86us) leaves room for ~3 packs of 0.43us each.
                Iterations carrying a projection wedge get 2.  Z packs only
                need et and the z banks (free after the Q-B wedge at it=8),
                so they start at it=9; AV packs need masks and V_sb, so
                they start at it=13.  23 iterations x ~3 packs >= 64 packs
                drains everything by the last iteration.  A pack may not
                enter a new qc until that qc's predecessor epilogue is out
                (its first matmul re-claims the banks the epilogue eviction
                reads)."""
                nonlocal z_next, av_next
                budget = 2 if (it in WEDGE_ITS or it % 4 == 2) else 3
                if it >= 30 or it >= NIT:
                    budget = 6
                zlim = it - 1 if it >= ZLAG - 1 else -1
                alim = it - 3 if it >= AVLAG else -1
                if it >= 29:
                    alim = it
                while budget > 0:
                    jz, ja = z_next, av_next
                    z_ok = (
                        jz < NIT
                        and jz <= zlim
                        and not (jz % NKT == 0 and jz > 0 and epi_done < jz // NKT)
                    )
                    a_ok = (
                        ja < NIT
                        and ja <= alim
                        and not (ja % NKT == 0 and ja > 0 and epi_done < ja // NKT)
                    )
                    if not z_ok and not a_ok:
                        break
                    # alternate, preferring whichever stream is furthest behind
                    if z_ok and (not a_ok or jz - ja <= 4):
                        emit_z_pack(jz)
                        z_next += 1
                    else:
                        emit_av_pack(ja)
                        av_next += 1
                    budget -= 1
                    try_epi()

            for it in range(NIT):
                last = it == NIT - 1
                if it + MLEAD < NIT:
                    maskp_tiles[it + MLEAD] = emit_mask_dma(it + MLEAD)

                # deferred projections, wedged at the TOP of the body so
                # their PSUM->SBUF evictions sit in the DVE FIFO ahead of
                # this iteration's (possibly mask-gated) A^T multiplies.
                # Each wedge's inputs are in SBUF just before the PE's FIFO
                # reaches it, so it never blocks the S matmuls behind it.
                if it == 1:
                    emit_proj(KT_sb, wk_sb, bqk_sb[:, 1:2], ka2_t, 0,
                              agg_tag_tile("ps_kA2b"),
                              [(HF - KA0, QC - KA0, HF)])
                elif it == 4:
                    emit_proj(KT_sb, wk_sb, bqk_sb[:, 1:2], kb_t, 1,
                              z_tag_tile("ps_kB", shape=(P, QC)),
                              [(0, HF, 0), (HF, QC, HF)])
                elif it == 8:
                    emit_proj(QT_sb, wq_sb, bqk_sb[:, 0:1], qb_t, 1,
                              z_tag_tile("ps_qB", shape=(P, QC)),
                              [(0, HF, 0), (HF, QC, HF)])
                elif it == 9:
                    v_ps[0] = agg_tag_tile("ps_vA")
                    emit_v_chunk(0, v_ps[0])
                elif it == 10:
                    emit_v_chunk(1, v_ps[0])
                elif it == 11:
                    v_ps[1] = agg_tag_tile("ps_vB")
                    emit_v_chunk(2, v_ps[1])
                elif it == 12:
                    emit_v_chunk(3, v_ps[1])

                mt = maskp_tiles.pop(it)
                et = etp.tile([P, HPC, QC], BF16, tag="et", name=f"et_{it}")
                at = atp.tile([P, HPC, QC], BF16, tag="at", name=f"at_{it}")
                ets[it], ats[it] = et, at
                s_nxt = (
                    [s_tile(lh, f"s_{it + 1}_{lh}") for lh in range(HPC)]
                    if not last
                    else None
                )
                for lh in range(HPC):
                    nc.scalar.activation(
                        et[:, lh, :],
                        s_cur[lh],
                        mybir.ActivationFunctionType.Exp,
                        scale=SCALING,
                    )
                    nc.vector.tensor_mul(at[:, lh, :], et[:, lh, :], mt[:, lh, :])
                    # S^T for the next iteration reuses this head's PSUM
                    # banks; emit right after the exp that frees them.
                    if not last:
                        emit_s(it + 1, lh, s_nxt[lh])
                s_cur = s_nxt
                drain_packs(it)

            it = NIT
            while z_next < NIT or av_next < NIT:
                drain_packs(it)
                it += 1

    nc.compile()
    return nc


# ---------------------------------------------------------------------------
# Host side
# ---------------------------------------------------------------------------
def _prep_in_maps(q, k, v, mask_head, pearson_matrix, Wq, bq, Wk, bk, Wv, bv):
    f = np.float32
    q = np.asarray(q, f)
    k = np.asarray(k, f)
    v = np.asarray(v, f)
    mask_head = np.asarray(mask_head, f)
    Wq = np.asarray(Wq, f)
    Wk = np.asarray(Wk, f)
    Wv = np.asarray(Wv, f)
    bq = np.asarray(bq, f).reshape(D)
    bk = np.asarray(bk, f).reshape(D)
    bv = np.asarray(bv, f).reshape(D)

    # Only the diagonal of pearson is used by the computation.
    pm = np.asarray(pearson_matrix)
    diag = np.ascontiguousarray(np.diagonal(pm, axis1=-2, axis2=-1)).astype(f)

    def _ptile(x, lo, hi):
        """x [n, d] -> [p, c, hi-lo] bf16 with d = c*128 + p, cols lo:hi of
        the transposed matrix; per-partition contiguous."""
        dst = _alloc((P, NCC, hi - lo), NPBF16)
        xT = x.T.reshape(NCC, P, x.shape[0])  # [c, p, n]
        np.copyto(dst, xT[:, :, lo:hi].transpose(1, 0, 2))
        return dst

    qT = [[_ptile(q[b], 0, QC), _ptile(q[b], QC, N)] for b in range(B)]
    kT = [
        [_ptile(k[b], 0, KA0), _ptile(k[b], KA0, QC), _ptile(k[b], QC, N)]
        for b in range(B)
    ]
    vT = [[_ptile(v[b], 0, QC), _ptile(v[b], QC, N)] for b in range(B)]
    onesd = np.ones((P, P), NPBF16)

    def wtile(W, esl):
        # [D, E] -> [P, NCC, E] with d = c*P + p
        wT = np.ascontiguousarray(W[esl, :].T.astype(NPBF16))
        return np.ascontiguousarray(wT.reshape(NCC, P, E).transpose(1, 0, 2))

    # Per-(b,h) mask, transposed to [k, q], diag-folded, tiled to the exact
    # per-iteration consumption order: [qc, kt, k, lh, q].
    maskt_all = _alloc((B, H // HPC, NQC, NKT, P, HPC, QC), NPBF16)
    for b in range(B):
        for h in range(H):
            md = mask_head[b, h].T * diag[b, h][:, None]  # [k, q] f32
            tiled = md.reshape(NKT, P, NQC, QC).transpose(2, 0, 1, 3)
            maskt_all[b, h // HPC, :, :, :, h % HPC, :] = tiled

    in_maps = []
    for c in range(NCORES):
        b = c // (NCORES // B)
        h0 = HPC * (c % (NCORES // B))
        esl = slice(h0 * HD, (h0 + HPC) * HD)
        bqk_h = np.ascontiguousarray(
            np.stack([bq[esl], bk[esl]], axis=1).astype(f)
        )
        bvb_h = np.ascontiguousarray(
            np.broadcast_to(bv[esl][None, :], (P, E)).astype(f)
        )
        in_maps.append(
            {
                "qah": qT[b][0],
                "qbh": qT[b][1],
                "ka1h": kT[b][0],
                "ka2h": kT[b][1],
                "kbh": kT[b][2],
                "v0h": vT[b][0],
                "v1h": vT[b][1],
                "wq": wtile(Wq, esl),
                "wk": wtile(Wk, esl),
                "wv": wtile(Wv, esl),
                "bqk": bqk_h,
                "bvb": bvb_h,
                "onesd": onesd,
                "maskt": maskt_all[b, h0 // HPC],
            }
        )
    return in_maps


_NC_CACHE = None
LAST_RESULT = None  # BassKernelResults of the most recent run (for profiling)


def kernel(**inputs) -> np.ndarray:
    global _NC_CACHE, LAST_RESULT
    _install_shims()
    from concourse.bass_utils import run_bass_kernel_spmd

    if _NC_CACHE is None:
        _NC_CACHE = build_nc()
    nc = _NC_CACHE

    in_maps = _prep_in_maps(**inputs)

    trace = bool(int(os.environ.get("KERNEL_TRACE", "0")))
    kwargs = {}
    if trace:
        kwargs["trace"] = True
        tmpdir = os.environ.get("KERNEL_TRACE_DIR")
        if tmpdir:
            kwargs["tmpdir"] = tmpdir
    res = run_bass_kernel_spmd(nc, in_maps, list(range(NCORES)), **kwargs)
    LAST_RESULT = res

    out = _alloc((B, N, D), np.float32)
    for c in range(NCORES):
        b = c // (NCORES // B)
        h0 = HPC * (c % (NCORES // B))
        aggT = np.asarray(res.results[c]["outT"], np.float32)  # (E, N)
        z = np.asarray(res.results[c]["zout"], np.float32)  # (HPC, N)
        out[b, :, h0 * HD : (h0 + HPC) * HD] = (
            aggT / np.repeat(z, HD, axis=0)
        ).T
    return out
